# revision 2
# baseline (speedup 1.0000x reference)
"""Trainium2 Bass kernel for MultiHeadSelfAttention (GroupNorm + QKV +
attention + proj + residual). fp8 DoubleRow edition, ~106.6us (was 130.2us).

Problem shape (hardcoded): x [8, 512, 32, 32] fp32, 8 heads, 32 groups.
Sharding: data-parallel over batch B=8 across the 8 NeuronCores.

Key structure vs the f32r baseline:
  - QK^T runs as fp8e4 DoubleRow matmuls with the head's 64 channels split
    [32 partitions x 2 k-tiles] (0.5 cyc/row on PE). q/k quantized from the
    qkv psum by DVE (bias folded into the quantize). Heads 3 and 7 would
    need base partition 96 (illegal), so their units are relocated to fix
    tiles FK/FQ by SBUF->SBUF DMA.
  - softmax exp on ACT in [128,1024] tiles writing fp8 directly; the 1/8
    logit scale is applied via the activation scale field. ACT is the
    critical engine (~66us of exp); everything else is scheduled under it.
  - AV runs as fp8 DoubleRow with s-pair k-tiles; vt tiles carry a ones
    rider column producing softmax row sums in psum partition 0.
  - v bias is folded into the proj bias on the host (proj(a+bv) =
    proj(a) + proj_w@bv + proj_b).
  - proj in f32r in 3 passes (pairs 0+1, 2, 3) to shorten the tail.
"""

import ml_dtypes
import numpy as np

import concourse.bass as bass
import concourse.bacc as bacc
import concourse.tile as tile
import concourse.mybir as mybir
from concourse import library_config
from concourse.bass_utils import run_bass_kernel_spmd

B, C, HS, WS = 8, 512, 32, 32
T = HS * WS            # 1024
H = 8                  # heads
CH = C // H            # 64
G = 32                 # groups
CPG = C // G           # 16 channels per group
EPS = 1e-5
NCHUNK = C // 128      # 4 channel chunks
NT = T // 128          # 8 sequence tiles
NB = T // 512          # 2 psum banks over T
NSP = NT // 2          # 4 s-tile pairs
F32 = mybir.dt.float32
F32R = mybir.dt.float32r
FP8 = mybir.dt.float8e4
DR = mybir.MatmulPerfMode.DoubleRow
E4 = ml_dtypes.float8_e4m3

_CACHE = {}
_DEBUG = False


def _orig_row(kind, h, i):
    off = {"q": 0, "k": CH, "v": 2 * CH}[kind]
    return 192 * h + off + i


def _host_weights(gn_w, gn_b, qkv_w, qkv_b, proj_w, proj_b):
    # qkgen m-tile row mapping: m 0,1 = k(j0,j1) heads 0-3; 2,3 = q heads
    # 0-3; 4,5 = k heads 4-7; 6,7 = q heads 4-7. partition u = 32*(h%4) +
    # (c%32), j = c//32.
    rows = np.zeros((8, 128), np.int64)
    for m in range(8):
        grp = m // 4
        kind = "k" if (m % 4) < 2 else "q"
        j = m % 2
        for u in range(128):
            h = 4 * grp + u // 32
            c = 32 * j + u % 32
            rows[m, u] = _orig_row(kind, h, c)
    wqk = qkv_w[rows.reshape(-1), :].T.copy()                # [512, 1024]
    wqk_t = np.ascontiguousarray(
        wqk.reshape(NCHUNK, 128, 1024)).astype(ml_dtypes.bfloat16)
    bqk = qkv_b[rows].T.copy().astype(np.float32)            # [128, 8]

    vrows = np.array([_orig_row("v", h, i) for h in range(H) for i in range(CH)])
    wv = qkv_w[vrows, :].T.copy()                            # [512(c), 512(cv)]
    wv_t = np.ascontiguousarray(wv.reshape(NCHUNK, 128, C)).astype(ml_dtypes.bfloat16)
    bv_vec = qkv_b[vrows]                                    # [512] in a-order

    wproj = proj_w.T.copy()                                  # [512(c), 512(o)]
    wproj_t = np.ascontiguousarray(wproj.reshape(NCHUNK, 128, C)).astype(ml_dtypes.bfloat16)
    bproj_eff = proj_b + proj_w @ bv_vec                     # fold v bias
    bproj = bproj_eff.reshape(NCHUNK, 128).T.copy().astype(np.float32)

    gnw = gn_w.reshape(NCHUNK, 128).T.copy()                 # [128, 4]
    gnb = gn_b.reshape(NCHUNK, 128).T.copy()

    # per-chunk group combine: same pattern for every chunk
    g_k = np.zeros((128, 8), dtype=np.float32)
    gt_k = np.zeros((8, 128), dtype=np.float32)
    for u in range(128):
        g_k[u, u // CPG] = 1.0 / CPG
        gt_k[u // CPG, u] = 1.0
    riderpad = np.zeros((128, 64, 64), dtype=np.float32)
    riderpad[:, :, 0] = 1.0
    consts1 = np.concatenate([gnw, gnb, bqk, bproj], axis=1)  # [128, 20]
    consts2 = np.zeros((128, 136), dtype=np.float32)
    consts2[:, 0:8] = g_k
    consts2[0:8, 8:136] = gt_k
    return {
        "x": None,
        "wqk": np.ascontiguousarray(wqk_t.transpose(1, 0, 2)),
        "wv": np.ascontiguousarray(wv_t.transpose(1, 0, 2)),
        "wproj": np.ascontiguousarray(wproj_t.transpose(1, 0, 2)),
        "consts1": consts1, "consts2": consts2,
        "riderpad": riderpad.astype(ml_dtypes.float8_e4m3),
    }


def _build_program():
    nc = bacc.Bacc("TRN2", target_bir_lowering=False, debug=False, num_devices=8)
    BF16 = mybir.dt.bfloat16
    dt_in = [
        ("x", [128, NCHUNK, T], BF16), ("wqk", [128, NCHUNK, 1024], BF16),
        ("wv", [128, NCHUNK, C], BF16), ("wproj", [128, NCHUNK, C], BF16),
        ("consts1", [128, 20], F32), ("consts2", [128, 136], F32R),
        ("riderpad", [128, 64, 64], FP8),
    ]
    d = {name: nc.dram_tensor(name, shape, dt, kind="ExternalInput").ap()
         for name, shape, dt in dt_in}
    out_d = nc.dram_tensor("out", [C, T], BF16, kind="ExternalOutput").ap()

    with tile.TileContext(nc) as tc:
        with (
            tc.tile_pool(name="singles", bufs=1) as singles,
            tc.tile_pool(name="small", bufs=10) as small,
            tc.tile_pool(name="ewp", bufs=3) as ewp,
            tc.tile_pool(name="recp", bufs=2) as recp,
            tc.tile_pool(name="rbp", bufs=2) as rbp,
            tc.tile_pool(name="pA", bufs=2, space="PSUM") as pA,
            tc.tile_pool(name="pB", bufs=2, space="PSUM") as pB,
            tc.tile_pool(name="pC", bufs=1, space="PSUM") as pC,
        ):
            # ---- consolidated input DMAs (HWDGE costs ~630ns per DMA,
            # ---- serially -- few big transfers beat many small ones) ----
            x_all = singles.tile([128, NCHUNK, T], BF16, tag="x_all")
            nc.sync.dma_start(x_all[:], d["x"][:])
            x_sb = [x_all[:, k, :] for k in range(NCHUNK)]
            consts1 = singles.tile([128, 20], F32, tag="consts1")
            nc.scalar.dma_start(consts1[:], d["consts1"][:])
            consts2 = singles.tile([128, 136], F32R, tag="consts2")
            nc.sync.dma_start(consts2[:], d["consts2"][:])
            gnw_sb, gnb_sb = consts1[:, 0:4], consts1[:, 4:8]
            bqk_sb, bproj_sb = consts1[:, 8:16], consts1[:, 16:20]
            g_sb, gt_sb = consts2[:, 0:8], consts2[0:8, 8:136]
            wqk_all = singles.tile([128, NCHUNK, 1024], BF16, tag="wqk_all")
            nc.sync.dma_start(wqk_all[:], d["wqk"][:])
            wqk_sb = [wqk_all[:, k, :] for k in range(NCHUNK)]
            wv_all = singles.tile([128, NCHUNK, C], BF16, tag="wv_all")
            nc.sync.dma_start(wv_all[:], d["wv"][:])
            wv_sb = [wv_all[:, k, :] for k in range(NCHUNK)]
            wproj_all = singles.tile([128, NCHUNK, C], BF16, tag="wproj_all")
            nc.sync.dma_start(wproj_all[:], d["wproj"][:])
            wproj_sb = [wproj_all[:, k, :] for k in range(NCHUNK)]
            nc.gpsimd.load_library(library_config.attn)
            eps_t = singles.tile([8, 1], F32, tag="eps")
            nc.vector.memset(eps_t[:], EPS)
            zero_t = singles.tile([128, 1], F32, tag="zero_t")
            nc.vector.memset(zero_t[:], 0.0)

            # ---- PE warm-up during the x DMA (pstate ramp) ----
            wz = singles.tile([128, 512], BF16, tag="wz")
            nc.vector.memset(wz[:], 0.0)
            for _ in range(16):
                pwz = pB.tile([128, 512], F32, tag="pB", name="pwz")
                nc.tensor.matmul(pwz[:], wz[:, 0:128], wz[:],
                                 start=True, stop=True)

            # ---- fp8 stage tiles ----
            tK = [singles.tile([128, 2, T], FP8, tag=f"tK{i}", name=f"tK{i}")
                  for i in range(2)]
            tQ = [singles.tile([128, 2, T], FP8, tag=f"tQ{i}", name=f"tQ{i}")
                  for i in range(2)]
            FK = singles.tile([64, 2, T], FP8, tag="FK")
            FQ = singles.tile([64, 2, T], FP8, tag="FQ")
            vt8_all = singles.tile([128, NSP, 2, H, 128], FP8, tag="vt8_all")
            nc.scalar.dma_start(
                vt8_all[:].rearrange("p sp j h c -> p (sp j h) c")[:, :, 0:64],
                d["riderpad"][:])
            vt8 = [vt8_all[:, sp] for sp in range(NSP)]

            a_sb = [singles.tile([128, T], BF16, tag=f"a{p}", name=f"a{p}")
                    for p in range(NCHUNK)]
            acc_sb = [singles.tile([128, T], BF16, tag=f"acc{m}", name=f"acc{m}")
                      for m in range(NCHUNK)]

            # ---- GroupNorm, engine-stage-batched across chunks ----
            # (per-chunk emission serializes on cross-engine hops; batching
            # stages pipelines all four chunks through each engine once)
            mv_l, stats_l, gsb_l, varg_l, rstd_l, grp_l = [], [], [], [], [], []
            for k in range(NCHUNK):
                st6 = small.tile([128, 2, 6], F32, tag="small", name=f"st6_{k}")
                nc.vector.bn_stats(st6[:, 0, :], x_sb[k][:, 0:512])
                nc.vector.bn_stats(st6[:, 1, :], x_sb[k][:, 512:1024])
                mv = small.tile([128, 2], F32, tag="small", name=f"mv{k}")
                nc.vector.bn_aggr(mv[:], st6[:])
                mv_l.append(mv)
            for k in range(NCHUNK):
                m2 = small.tile([128, 1], F32, tag="small", name=f"m2_{k}")
                nc.vector.tensor_mul(m2[:], mv_l[k][:, 0:1], mv_l[k][:, 0:1])
                stats = small.tile([128, 2], F32R, tag="small", name=f"sts{k}")
                nc.vector.tensor_copy(stats[:, 0:1], mv_l[k][:, 0:1])
                nc.vector.tensor_add(stats[:, 1:2], mv_l[k][:, 1:2], m2[:])
                stats_l.append(stats)
            psg_l = []
            for k in range(NCHUNK):
                psum_g = pB.tile([8, 2], F32, tag="pB", name=f"psg{k}")
                nc.tensor.matmul(psum_g[:], g_sb[:], stats_l[k][:],
                                 start=True, stop=True)
                psg_l.append(psum_g)
            for k in range(NCHUNK):
                gsb = small.tile([8, 2], F32, tag="small", name=f"gsb{k}")
                nc.vector.tensor_copy(gsb[:], psg_l[k][:])
                mu2 = small.tile([8, 1], F32, tag="small", name=f"mu2_{k}")
                nc.vector.tensor_mul(mu2[:], gsb[:, 0:1], gsb[:, 0:1])
                varg = small.tile([8, 1], F32, tag="small", name=f"vg{k}")
                nc.vector.tensor_sub(varg[:], gsb[:, 1:2], mu2[:])
                gsb_l.append(gsb); varg_l.append(varg)
            sdv_l = []
            for k in range(NCHUNK):
                sdv = small.tile([8, 1], F32, tag="small", name=f"sdv{k}")
                nc.scalar.activation(sdv[:], varg_l[k][:],
                                     mybir.ActivationFunctionType.Sqrt,
                                     bias=eps_t[:], scale=1.0)
                sdv_l.append(sdv)
            for k in range(NCHUNK):
                rstd = small.tile([8, 1], F32, tag="small", name=f"rst{k}")
                nc.vector.reciprocal(rstd[:], sdv_l[k][:])
                grp = small.tile([8, 2], F32R, tag="small", name=f"grp{k}")
                nc.vector.tensor_copy(grp[:, 0:1], gsb_l[k][:, 0:1])
                nc.vector.tensor_copy(grp[:, 1:2], rstd[:])
                grp_l.append(grp)
            psc_l = []
            for k in range(NCHUNK):
                psum_pc = pB.tile([128, 2], F32, tag="pB", name=f"psc{k}")
                nc.tensor.matmul(psum_pc[:], gt_sb[:], grp_l[k][:],
                                 start=True, stop=True)
                psc_l.append(psum_pc)
            h_sb = []
            sc_l, bc_l = [], []
            for k in range(NCHUNK):
                s_c = small.tile([128, 1], F32, tag="small", name=f"s_c{k}")
                nc.vector.tensor_mul(s_c[:], psc_l[k][:, 1:2], gnw_sb[:, k:k + 1])
                t1 = small.tile([128, 1], F32, tag="small", name=f"t1_{k}")
                nc.vector.tensor_mul(t1[:], psc_l[k][:, 0:1], s_c[:])
                b_c = small.tile([128, 1], F32, tag="small", name=f"b_c{k}")
                nc.vector.tensor_sub(b_c[:], gnb_sb[:, k:k + 1], t1[:])
                sc_l.append(s_c); bc_l.append(b_c)
            for k in range(NCHUNK):
                ht = singles.tile([128, T], BF16, tag=f"h{k}")
                if k % 2 == 0:
                    nc.scalar.activation(
                        ht[:], x_sb[k][:],
                        mybir.ActivationFunctionType.Identity,
                        bias=bc_l[k][:], scale=sc_l[k][:])
                else:
                    nc.vector.tensor_scalar(
                        out=ht[:], in0=x_sb[k][:], scalar1=sc_l[k][:],
                        scalar2=bc_l[k][:], op0=mybir.AluOpType.mult,
                        op1=mybir.AluOpType.add)
                h_sb.append(ht)

            # ---- qkgen (m, nb) quarter: 4 matmuls + fp8 quantize ----
            # [128,512] quarters live in the fast pB ring so the slow
            # exp-paced pw ring (pA) never blocks weight generation.
            def emit_qkgen_q(m, nb, on_act=False):
                pq = pB.tile([128, 512], F32, tag="pB", name="pq")
                for k in range(NCHUNK):
                    nc.tensor.matmul(
                        pq[:],
                        wqk_sb[k][:, 128 * m:128 * (m + 1)],
                        h_sb[k][:, 512 * nb:512 * (nb + 1)],
                        start=(k == 0), stop=(k == 3))
                dst = (tK if (m % 4) < 2 else tQ)[m // 4]
                out_ap = dst[:, m % 2, 512 * nb:512 * (nb + 1)]
                if on_act:
                    nc.scalar.activation(
                        out_ap, pq[:], mybir.ActivationFunctionType.Identity,
                        bias=bqk_sb[:, m:m + 1])
                else:
                    nc.vector.tensor_scalar(
                        out=out_ap, in0=pq[:],
                        scalar1=bqk_sb[:, m:m + 1], scalar2=None,
                        op0=mybir.AluOpType.add)

            def emit_qkgen(m):
                for nb in range(NB):
                    emit_qkgen_q(m, nb)

            # ---- vgen t-tile: psum matmuls + fp8 quantize into vt8 ----
            def emit_vgen(mt):
                pv = pB.tile([128, 512], F32, tag="pB", name="pv")
                for k in range(NCHUNK):
                    nc.tensor.matmul(pv[:],
                                     h_sb[k][:, 128 * mt:128 * (mt + 1)],
                                     wv_sb[k][:], start=(k == 0), stop=(k == 3))
                nc.vector.tensor_scalar(
                    out=vt8[mt // 2][:, mt % 2, :, 64:128],
                    in0=pv[:].rearrange("p (h c) -> p h c", h=H),
                    scalar1=zero_t[:], scalar2=None,
                    op0=mybir.AluOpType.add)

            for m in range(4):
                emit_qkgen_q(m, 0)
            emit_qkgen_q(2, 1)
            emit_qkgen_q(3, 1)
            emit_qkgen_q(0, 1)
            emit_qkgen_q(1, 1)
            # head 3 unit relocation (base partition 96 is unaddressable)
            nc.sync.dma_start(FK[0:32, :, :], tK[0][96:128, :, :])
            nc.sync.dma_start(FQ[0:32, :, :], tQ[0][96:128, :, :])
            emit_vgen(0)
            emit_vgen(1)

            def vgen_duty():
                for mt in range(2, NT):
                    emit_vgen(mt)
                    yield

            def qkgen_duty():
                for m in range(4, 8):
                    for nb in range(NB):
                        emit_qkgen_q(m, nb)
                        yield
                nc.sync.dma_start(FK[32:64, :, :], tK[1][96:128, :, :])
                nc.sync.dma_start(FQ[32:64, :, :], tQ[1][96:128, :, :])
                yield

            # ---- proj pass: pairs `plist` into acc (stt on first pass) ----
            def proj_units(plist, first):
                for m in range(NCHUNK):
                    for nb in range(NB):
                        sl = slice(512 * nb, 512 * (nb + 1))
                        po = pB.tile([128, 512], F32, tag="pB", name="po")
                        for idx, p in enumerate(plist):
                            nc.tensor.matmul(
                                po[:], wproj_sb[p][:, 128 * m:128 * (m + 1)],
                                a_sb[p][:, sl], start=(idx == 0),
                                stop=(idx == len(plist) - 1))
                        if first:
                            nc.vector.scalar_tensor_tensor(
                                out=acc_sb[m][:, sl], in0=po[:],
                                scalar=bproj_sb[:, m:m + 1],
                                in1=x_sb[m][:, sl],
                                op0=mybir.AluOpType.add,
                                op1=mybir.AluOpType.add)
                        else:
                            nc.vector.tensor_add(
                                acc_sb[m][:, sl], po[:], acc_sb[m][:, sl])
                        yield

            # ---- AV + softmax-normalize for one head ----
            def emit_av_norm(h, ew, split_nb=False):
                p, s = h // 2, h % 2
                pa = pC.tile([128, T], F32, tag="pC", name="pa")
                if split_nb:
                    recs, rbs = [], []
                    for nb in range(NB):
                        sl = slice(512 * nb, 512 * (nb + 1))
                        for sp in range(NSP):
                            nc.tensor.matmul(
                                pa[:, sl], vt8[sp][:, :, h, :],
                                ew[:, 2 * sp:2 * sp + 2, sl],
                                start=(sp == 0), stop=(sp == NSP - 1),
                                perf_mode=DR)
                        rec = recp.tile([1, 512], F32, tag="rec", name="rec")
                        nc.vector.reciprocal_approx_fast(
                            rec[:], pa[0:1, sl])
                        rb = rbp.tile([CH, 512], F32, tag="rb", name="rb")
                        nc.gpsimd.partition_broadcast(rb[:], rec[:])
                        recs.append(rec); rbs.append(rb)
                    for nb in range(NB):
                        sl = slice(512 * nb, 512 * (nb + 1))
                        nc.vector.tensor_mul(
                            a_sb[p][64 * s:64 * s + 64, sl],
                            pa[64:128, sl], rbs[nb][:])
                else:
                    for nb in range(NB):
                        sl = slice(512 * nb, 512 * (nb + 1))
                        for sp in range(NSP):
                            nc.tensor.matmul(
                                pa[:, sl], vt8[sp][:, :, h, :],
                                ew[:, 2 * sp:2 * sp + 2, sl],
                                start=(sp == 0), stop=(sp == NSP - 1),
                                perf_mode=DR)
                    rec = recp.tile([1, T], F32, tag="rec", name="rec")
                    nc.vector.reciprocal_approx_fast(rec[:], pa[0:1, :])
                    rb = rbp.tile([CH, T], F32, tag="rb", name="rb")
                    nc.gpsimd.partition_broadcast(rb[:], rec[:])
                    nc.vector.tensor_mul(a_sb[p][64 * s:64 * s + 64, :],
                                         pa[64:128, :], rb[:])

            # ---- attention heads ----
            def head_slices(h):
                i, a = h // 4, h % 4
                if a < 3:
                    return tK[i], tQ[i], 32 * a
                return FK, FQ, 32 * i

            duties = [vgen_duty(), qkgen_duty()]
            step = 0
            prev = None
            for h in range(H):
                kt, qt, ba = head_slices(h)
                ew = ewp.tile([128, NT, T], FP8, tag="ew", name=f"ew{h}")
                for st in range(NT):
                    pw = pA.tile([128, T], F32, tag="pA", name="pw")
                    for nb in range(NB):
                        nc.tensor.matmul(
                            pw[:, 512 * nb:512 * (nb + 1)],
                            kt[ba:ba + 32, :, 128 * st:128 * (st + 1)],
                            qt[ba:ba + 32, :, 512 * nb:512 * (nb + 1)],
                            start=True, stop=True, perf_mode=DR)
                    nc.scalar.activation(ew[:, st, :], pw[:],
                                         mybir.ActivationFunctionType.Exp,
                                         scale=0.125)
                    step += 1
                    if duties and step % 2 == 0:
                        try:
                            next(duties[0])
                        except StopIteration:
                            duties.pop(0)
                if prev is not None:
                    emit_av_norm(*prev)
                prev = (h, ew)
                # proj passes appended only after their a_sb inputs'
                # writes are emitted (deferred-AV ordering)
                if h == 4:
                    duties.append(proj_units([0, 1], True))
                if h == 6:
                    duties.append(proj_units([2], False))

            for g in duties:
                for _ in g:
                    pass
            emit_av_norm(prev[0], prev[1], split_nb=True)

            # ---- tail: pair-3 + per-m output DMA ----
            out_eng = [nc.sync, nc.scalar, nc.sync, nc.scalar]
            for m in range(NCHUNK):
                for nb in range(NB):
                    sl = slice(512 * nb, 512 * (nb + 1))
                    po = pB.tile([128, 512], F32, tag="pB", name="po")
                    nc.tensor.matmul(
                        po[:], wproj_sb[3][:, 128 * m:128 * (m + 1)],
                        a_sb[3][:, sl], start=True, stop=True)
                    nc.vector.tensor_add(acc_sb[m][:, sl], po[:],
                                         acc_sb[m][:, sl])
                out_eng[m].dma_start(out_d[128 * m:128 * (m + 1), :],
                                     acc_sb[m][:])

    nc.compile()
    return nc


def _get_program(_n_reps=1):
    key = ("prog_v2",)
    if key not in _CACHE:
        _CACHE[key] = _build_program()
    return _CACHE[key]


def kernel(x, gn_w, gn_b, qkv_w, qkv_b, proj_w, proj_b, _n_reps=1):
    x = np.asarray(x, dtype=np.float32)
    hw = _host_weights(np.asarray(gn_w, np.float32), np.asarray(gn_b, np.float32),
                       np.asarray(qkv_w, np.float32), np.asarray(qkv_b, np.float32),
                       np.asarray(proj_w, np.float32), np.asarray(proj_b, np.float32))
    xr = np.ascontiguousarray(
        x.reshape(B, NCHUNK, 128, T).transpose(0, 2, 1, 3)).astype(
            ml_dtypes.bfloat16)
    nc = _get_program()
    hw = {k: v for k, v in hw.items() if v is not None}
    in_maps = [dict(hw, x=xr[b]) for b in range(B)]
    res = run_bass_kernel_spmd(nc, in_maps, core_ids=list(range(B)))
    out = np.stack([np.asarray(res.results[b]["out"]).astype(np.float32)
                    for b in range(B)])
    return out.reshape(B, C, HS, WS)
